# revision 1
# baseline (speedup 1.0000x reference)
"""Trainium2 Bass kernel for AttributeAttentionModule.

y = attention over heads of QKV projections:
  Q = sa @ Wq.T + bq ; K = x @ Wk.T + bk ; V = x @ Wv.T + bv   (all [B, D])
  per-sample scores[h,g] = Q_h . K_g / 32 ; softmax over g ; out_h = sum_g w_hg V_g

Data-parallel over 8 NeuronCores (batch sharded). Matmuls run in float32r
(FP22) at 1 cycle/row. Weights are streamed once per group of 8 batch-tiles
(all 8 PSUM banks accumulate in parallel over the contraction dim). All HBM
operands are pre-tiled on the host so every DMA descriptor is a contiguous
12KB-per-partition block. Attention is software-pipelined into the matmul
stream via filler chunks emitted after each o-sweep's PSUM copies.
"""

import os
import sys

for _p in ("/opt/trn_rl_repo", "/root/.axon_site/_ro/trn_rl_repo"):
    if os.path.isdir(_p) and _p not in sys.path:
        sys.path.append(_p)

import numpy as np
from contextlib import ExitStack

B = 16384
D = 3072
H = 3
DH = D // H          # 1024
NCORES = 8
P = 128              # partition tile
NO = 512             # matmul moving free dim (one PSUM bank of fp32)
KGRP = 3             # k-tiles per weight DMA
KT = D // P          # 24 contraction tiles
NOT = D // NO        # 6 output-column tiles
NKG = KT // KGRP     # 4 weight DMAs per o-column
KHALF = KT // 2      # stationary tiles arrive in two halves

_CACHE = {}


def _build(bs=B // NCORES, gbt=8):
    """Build + compile the per-core program. bs = batch rows per core,
    gbt = batch tiles (of 128) per weight-streaming group."""
    import concourse.bass as bass
    import concourse.tile as tile
    from concourse import bacc, mybir

    f32 = mybir.dt.float32
    f32r = mybir.dt.float32r
    mult = mybir.AluOpType.mult
    add = mybir.AluOpType.add
    bypass = mybir.AluOpType.bypass
    Exp = mybir.ActivationFunctionType.Exp

    nbt = bs // P        # batch tiles per core
    ng = nbt // gbt      # weight-stream groups

    nc = bacc.Bacc(
        "TRN2", target_bir_lowering=False, debug=False, num_devices=NCORES
    )

    # pre-tiled inputs (see kernel() for host layouts)
    sa4 = nc.dram_tensor("sa4", [nbt, P, KT, P], f32r, kind="ExternalInput").ap()
    x4 = nc.dram_tensor("x4", [nbt, P, KT, P], f32r, kind="ExternalInput").ap()
    wT = {
        t: nc.dram_tensor(
            f"w{t}5", [NOT, NKG, P, KGRP, NO], f32r, kind="ExternalInput"
        ).ap()
        for t in "qkv"
    }
    biasd = {
        t: nc.dram_tensor(f"b{t}", [P, D], f32, kind="ExternalInput").ap()
        for t in "qkv"
    }
    outd = nc.dram_tensor("out", [bs, D], f32, kind="ExternalOutput").ap()

    with tile.TileContext(nc) as tc, ExitStack() as ctx:
        dram = ctx.enter_context(tc.tile_pool(name="dram", bufs=1, space="DRAM"))
        qkv_s = {t: dram.tile([bs, D], f32, tag=f"s{t}", name=f"s{t}") for t in "qkv"}

        apool = ctx.enter_context(tc.tile_pool(name="apool", bufs=1))
        wpool = ctx.enter_context(tc.tile_pool(name="wpool", bufs=4))
        bpool = ctx.enter_context(tc.tile_pool(name="bpool", bufs=1))
        ocpool = ctx.enter_context(tc.tile_pool(name="ocpool", bufs=3))
        pspool = ctx.enter_context(tc.tile_pool(name="psum", bufs=1, space="PSUM"))
        qkvp = ctx.enter_context(tc.tile_pool(name="qkvp", bufs=1))
        smallp = ctx.enter_context(tc.tile_pool(name="smallp", bufs=4))
        accp = ctx.enter_context(tc.tile_pool(name="accp", bufs=2))
        prodp = ctx.enter_context(tc.tile_pool(name="prodp", bufs=1))
        outp = ctx.enter_context(tc.tile_pool(name="outp", bufs=1))

        pending = []  # attention chunk closures, drained between o-sweeps

        def filler():
            if pending:
                pending.pop(0)()

        def load_act(src, g):
            """Two half-k tiles per batch tile so matmuls can start on the
            first half while the second streams in."""
            los, his = [], []
            for i in range(gbt):
                lo = apool.tile([P, KHALF, P], f32r, tag=f"a{i}l", name=f"a{i}l")
                nc.gpsimd.dma_start(lo[:], src[g * gbt + i, :, 0:KHALF, :])
                los.append(lo)
            for i in range(gbt):
                hi = apool.tile([P, KHALF, P], f32r, tag=f"a{i}h", name=f"a{i}h")
                nc.gpsimd.dma_start(hi[:], src[g * gbt + i, :, KHALF:KT, :])
                his.append(hi)
            return list(zip(los, his))

        def proj(items, wTd, bias_d, dst, first_o_fill=True):
            """items: list of (global_bt_index, (a_lo, a_hi))."""
            bias_t = bpool.tile([P, D], f32, tag="bias", name="bias")
            nc.sync.dma_start(bias_t[:], bias_d[:])
            for o in range(NOT):
                ps = {
                    bt: pspool.tile([P, NO], f32, tag=f"ps{j}", name=f"ps{j}")
                    for j, (bt, _) in enumerate(items)
                }
                for kg in range(NKG):
                    wt = wpool.tile([P, KGRP, NO], f32r, tag="w", name="w")
                    nc.gpsimd.dma_start(wt[:], wTd[o, kg])
                    for j in range(KGRP):
                        k = kg * KGRP + j
                        for bt, (alo, ahi) in items:
                            a = alo if k < KHALF else ahi
                            nc.tensor.matmul(
                                ps[bt][:],
                                a[:, k % KHALF, :],
                                wt[:, j, :],
                                start=(k == 0),
                                stop=(k == KT - 1),
                            )
                for bt, _ in items:
                    oc = ocpool.tile([P, NO], f32, tag="oc", name="oc")
                    nc.vector.tensor_add(
                        oc[:], ps[bt][:], bias_t[:, o * NO : (o + 1) * NO]
                    )
                    nc.scalar.dma_start(
                        dst[bt * P : bt * P + P, o * NO : (o + 1) * NO], oc[:]
                    )
                if first_o_fill or o > 0:
                    filler()

        def attn_chunks(bt):
            """Two closures per batch tile: A = load + scores + softmax,
            B = weighted V combine + store."""
            r0 = bt * P
            t3 = {}
            small = {}

            def chunk_a():
                for t in "qkv":
                    tt = qkvp.tile([P, D], f32, tag=t, name=f"t_{t}")
                    nc.scalar.dma_start(tt[:], qkv_s[t][r0 : r0 + P, :])
                    t3[t] = tt
                s = smallp.tile([P, H * H], f32, tag="s", name="s")
                prod = prodp.tile([P, DH], f32, tag="prod", name="prod")
                for h in range(H):
                    for g2 in range(H):
                        # fused row-wise dot: prod = Q_h*K_g ; s_hg = sum(prod)
                        nc.vector.scalar_tensor_tensor(
                            prod[:],
                            t3["q"][:, h * DH : (h + 1) * DH],
                            1.0,
                            t3["k"][:, g2 * DH : (g2 + 1) * DH],
                            op0=bypass,
                            op1=mult,
                            accum_out=s[:, h * H + g2 : h * H + g2 + 1],
                        )
                e = smallp.tile([P, H * H], f32, tag="e", name="e")
                nc.scalar.activation(e[:], s[:], Exp, scale=1.0 / 32.0)
                ssum = smallp.tile([P, H], f32, tag="ssum", name="ssum")
                nc.vector.tensor_reduce(
                    ssum[:],
                    e[:].rearrange("p (h g) -> p h g", h=H),
                    axis=mybir.AxisListType.X,
                    op=add,
                )
                rcp = smallp.tile([P, H], f32, tag="rcp", name="rcp")
                nc.vector.reciprocal(rcp[:], ssum[:])
                small["e"] = e
                small["rcp"] = rcp

            def chunk_b():
                e, rcp = small["e"], small["rcp"]
                ot = outp.tile([P, D], f32, tag="o", name="o")
                for h in range(H):
                    acc = accp.tile([P, DH], f32, tag="acc", name="acc")
                    # first term on ScalarE (per-partition scalar scale)
                    nc.scalar.mul(acc[:], t3["v"][:, 0:DH], e[:, h * H : h * H + 1])
                    for g2 in (1, 2):
                        nc.vector.scalar_tensor_tensor(
                            acc[:],
                            t3["v"][:, g2 * DH : (g2 + 1) * DH],
                            e[:, h * H + g2 : h * H + g2 + 1],
                            acc[:],
                            op0=mult,
                            op1=add,
                        )
                    nc.scalar.mul(
                        ot[:, h * DH : (h + 1) * DH], acc[:], rcp[:, h : h + 1]
                    )
                nc.scalar.dma_start(outd[r0 : r0 + P, :], ot[:])

            return [chunk_a, chunk_b]

        for g in range(ng):
            last = g == ng - 1
            bts = [g * gbt + i for i in range(gbt)]
            sa_t = load_act(sa4, g)
            proj(list(zip(bts, sa_t)), wT["q"], biasd["q"], qkv_s["q"])
            x_t = load_act(x4, g)
            proj(list(zip(bts, x_t)), wT["k"], biasd["k"], qkv_s["k"])
            items = list(zip(bts, x_t))
            if last and gbt >= 2:
                half = gbt // 2
                proj(items[:half], wT["v"], biasd["v"], qkv_s["v"])
                for bt in bts[:half]:
                    pending.extend(attn_chunks(bt))
                proj(items[half:], wT["v"], biasd["v"], qkv_s["v"])
                for bt in bts[half:]:
                    pending.extend(attn_chunks(bt))
            else:
                proj(items, wT["v"], biasd["v"], qkv_s["v"])
                for bt in bts:
                    pending.extend(attn_chunks(bt))
        while pending:
            pending.pop(0)()

    nc.compile()
    return nc


def _get_nc(bs=B // NCORES, gbt=8):
    key = (bs, gbt)
    if key not in _CACHE:
        _CACHE[key] = _build(bs, gbt)
    return _CACHE[key]


def _prep_weights(Wq, Wk, Wv, bq, bk, bv):
    """Pre-tile weights: w5[o, kg, p, j, n] = W.T[(kg*KGRP+j)*P + p, o*NO + n]."""
    ws = {}
    for nm, W in (("q", Wq), ("k", Wk), ("v", Wv)):
        wt = np.asarray(W, dtype=np.float32).T  # [in, out]
        w5 = wt.reshape(NKG, KGRP, P, NOT, NO).transpose(3, 0, 2, 1, 4)
        ws[nm] = np.ascontiguousarray(w5)
    bb = {
        nm: np.ascontiguousarray(
            np.broadcast_to(np.asarray(b, dtype=np.float32), (P, D))
        )
        for nm, b in (("q", bq), ("k", bk), ("v", bv))
    }
    return ws, bb


def _prep_act(a, bs):
    """Pre-tile activations per core: a4[bt, p, ko, b] = a[bt*P + b, ko*P + p]."""
    nbt = bs // P
    a4 = a.reshape(nbt, P, KT, P).transpose(0, 3, 2, 1)
    return np.ascontiguousarray(a4)


def _in_maps(x, sa, ws, bb, bs):
    maps = []
    for c in range(NCORES):
        r0 = c * bs
        maps.append(
            {
                "sa4": _prep_act(sa[r0 : r0 + bs], bs),
                "x4": _prep_act(x[r0 : r0 + bs], bs),
                "wq5": ws["q"],
                "wk5": ws["k"],
                "wv5": ws["v"],
                "bq": bb["q"],
                "bk": bb["k"],
                "bv": bb["v"],
            }
        )
    return maps


def kernel(x, synthetic_attributes, Wq, bq, Wk, bk, Wv, bv, **_ignored):
    from concourse import bass_utils

    x = np.asarray(x, dtype=np.float32)
    sa = np.asarray(synthetic_attributes, dtype=np.float32)
    bs = x.shape[0] // NCORES

    ws, bb = _prep_weights(Wq, Wk, Wv, bq, bk, bv)
    nc = _get_nc(bs=bs)
    in_maps = _in_maps(x, sa, ws, bb, bs)

    res = bass_utils.run_bass_kernel_spmd(nc, in_maps, core_ids=list(range(NCORES)))
    out = np.concatenate([res.results[c]["out"] for c in range(NCORES)], axis=0)
    return out



# revision 2
# speedup vs baseline: 1.9133x; 1.9133x over previous
"""Trainium2 Bass kernel for AttributeAttentionModule.

y = attention over heads of QKV projections:
  Q = sa @ Wq.T + bq ; K = x @ Wk.T + bk ; V = x @ Wv.T + bv   (all [B, D])
  per-sample scores[h,g] = Q_h . K_g / 32 ; softmax over g ; out_h = sum_g w_hg V_g

Data-parallel over 8 NeuronCores (batch sharded). Q/K projections run in
fp8 e4m3 with DoubleRow perf mode (2 rows/cycle, 256-deep contraction per
instruction); V runs in bf16 (fp8 noise in V would pass straight through the
softmax-convex combination, Q/K noise is damped by it). Weights are prescaled
by 128 for fp8 (raw weights sit below e4m3's normal range); the 1/128^2 is
folded into the softmax exp scale. Attention is fully fused in SBUF: Q is
kept per group of 4 batch-tiles, scores are accumulated inline as each K
PSUM chunk lands, V is kept bf16 per group and combined into the output
right after its last column chunks, so Q/K/V never round-trip through DRAM.
"""

import os
import sys

for _p in ("/opt/trn_rl_repo", "/root/.axon_site/_ro/trn_rl_repo"):
    if os.path.isdir(_p) and _p not in sys.path:
        sys.path.append(_p)

import numpy as np
import ml_dtypes
from contextlib import ExitStack

B = 16384
D = 3072
H = 3
DH = D // H          # 1024
NCORES = 8
P = 128              # partition tile
NO = 512             # matmul moving free dim (one PSUM bank of fp32)
NOT = D // NO        # 6 output-column tiles
KT = D // P          # 24 contraction tiles of 128
K8 = KT // 2         # 12 fp8 DoubleRow contraction tiles of 256
WS = 128.0           # fp8 weight prescale (power of two, exact)
ESCALE = 1.0 / (32.0 * WS * WS)  # softmax exp scale: 1/sqrt(dh) / WS^2
GBT = 4              # batch tiles per weight-streaming group

E4 = ml_dtypes.float8_e4m3
BF = ml_dtypes.bfloat16

_CACHE = {}


def _build(bs=B // NCORES, gbt=GBT):
    import concourse.bass as bass
    import concourse.tile as tile
    from concourse import bacc, mybir

    f32 = mybir.dt.float32
    f8 = mybir.dt.float8e4
    bf16 = mybir.dt.bfloat16
    mult = mybir.AluOpType.mult
    add = mybir.AluOpType.add
    bypass = mybir.AluOpType.bypass
    Exp = mybir.ActivationFunctionType.Exp
    DR = mybir.MatmulPerfMode.DoubleRow

    nbt = bs // P        # batch tiles per core
    ng = nbt // gbt      # weight-stream groups

    nc = bacc.Bacc(
        "TRN2", target_bir_lowering=False, debug=False, num_devices=NCORES
    )

    # pre-tiled inputs (see kernel() for host layouts)
    sa8d = nc.dram_tensor("sa8", [nbt, P, K8, 2, P], f8, kind="ExternalInput").ap()
    x8d = nc.dram_tensor("x8", [nbt, P, K8, 2, P], f8, kind="ExternalInput").ap()
    x16d = nc.dram_tensor("x16", [nbt, P, KT, P], bf16, kind="ExternalInput").ap()
    wq8d = nc.dram_tensor("wq8", [NOT, 4, P, 3, 2, NO], f8, kind="ExternalInput").ap()
    wk8d = nc.dram_tensor("wk8", [NOT, 4, P, 3, 2, NO], f8, kind="ExternalInput").ap()
    wv16d = nc.dram_tensor("wv16", [NOT, 8, P, 3, NO], bf16, kind="ExternalInput").ap()
    bqd = nc.dram_tensor("bq128", [P, D], bf16, kind="ExternalInput").ap()
    bkd = nc.dram_tensor("bk128", [P, D], bf16, kind="ExternalInput").ap()
    bvd = nc.dram_tensor("bv", [P, D], bf16, kind="ExternalInput").ap()
    outd = nc.dram_tensor("out", [bs, D], f32, kind="ExternalOutput").ap()

    with tile.TileContext(nc) as tc, ExitStack() as ctx:
        sapool = ctx.enter_context(tc.tile_pool(name="sapool", bufs=1))
        x8pool = ctx.enter_context(tc.tile_pool(name="x8pool", bufs=1))
        x16pool = ctx.enter_context(tc.tile_pool(name="x16pool", bufs=1))
        qpool = ctx.enter_context(tc.tile_pool(name="qpool", bufs=1))
        vpool = ctx.enter_context(tc.tile_pool(name="vpool", bufs=1))
        wpool = ctx.enter_context(tc.tile_pool(name="wpool", bufs=4))
        bpool = ctx.enter_context(tc.tile_pool(name="bpool", bufs=1))
        pspool = ctx.enter_context(tc.tile_pool(name="psum", bufs=2, space="PSUM"))
        kocpool = ctx.enter_context(tc.tile_pool(name="kocp", bufs=3))
        prodpool = ctx.enter_context(tc.tile_pool(name="prodp", bufs=2))
        otpool = ctx.enter_context(tc.tile_pool(name="otp", bufs=3))
        smallp = ctx.enter_context(tc.tile_pool(name="smallp", bufs=1))

        # biases resident for the whole kernel
        bias_t = {}
        for nm, src in (("q", bqd), ("k", bkd), ("v", bvd)):
            t = bpool.tile([P, D], bf16, tag=f"b{nm}", name=f"b{nm}")
            nc.sync.dma_start(t[:], src[:])
            bias_t[nm] = t

        for g in range(ng):
            bts = [g * gbt + i for i in range(gbt)]

            # activation loads for this group (sync queue: independent of
            # the weight stream on gpsimd so they land during prior passes)
            saT, x8T, x16T = [], [], []
            for i, bt in enumerate(bts):
                t = sapool.tile([P, K8, 2, P], f8, tag=f"sa{i}", name=f"sa{i}")
                nc.sync.dma_start(t[:], sa8d[bt])
                saT.append(t)
            for i, bt in enumerate(bts):
                t = x8pool.tile([P, K8, 2, P], f8, tag=f"x8{i}", name=f"x8{i}")
                nc.sync.dma_start(t[:], x8d[bt])
                x8T.append(t)
            for i, bt in enumerate(bts):
                t = x16pool.tile([P, KT, P], bf16, tag=f"x16{i}", name=f"x16{i}")
                nc.sync.dma_start(t[:], x16d[bt])
                x16T.append(t)

            qt = [
                qpool.tile([P, D], bf16, tag=f"q{i}", name=f"q{i}")
                for i in range(gbt)
            ]
            vt = [
                vpool.tile([P, D], bf16, tag=f"v{i}", name=f"v{i}")
                for i in range(gbt)
            ]
            s2 = [
                smallp.tile([P, 18], f32, tag=f"s2_{i}", name=f"s2_{i}")
                for i in range(gbt)
            ]
            en = [
                smallp.tile([P, 9], f32, tag=f"en_{i}", name=f"en_{i}")
                for i in range(gbt)
            ]

            def fp8_pass(wTd, acts, sink):
                """One fp8 DoubleRow projection sweep; sink(i, o, ps) consumes
                each finished PSUM chunk."""
                for o in range(NOT):
                    ps = [
                        pspool.tile([P, NO], f32, tag=f"ps{i}", name=f"ps{i}")
                        for i in range(gbt)
                    ]
                    for kg in range(4):
                        wt = wpool.tile([P, 3, 2, NO], f8, tag="w8", name="w8")
                        nc.gpsimd.dma_start(wt[:], wTd[o, kg])
                        for j in range(3):
                            k8 = kg * 3 + j
                            for i in range(gbt):
                                nc.tensor.matmul(
                                    ps[i][:],
                                    acts[i][:, k8],
                                    wt[:, j],
                                    start=(k8 == 0),
                                    stop=(k8 == K8 - 1),
                                    perf_mode=DR,
                                )
                    for i in range(gbt):
                        sink(i, o, ps[i])

            # ---- Q pass: oc = ps + 128*bq -> qt (bf16) ----
            def q_sink(i, o, ps):
                nc.vector.tensor_add(
                    qt[i][:, o * NO : (o + 1) * NO],
                    ps[:],
                    bias_t["q"][:, o * NO : (o + 1) * NO],
                )

            fp8_pass(wq8d, saT, q_sink)

            # ---- K pass: inline score partials, K never stored ----
            def k_sink(i, o, ps):
                g2, c = divmod(o, 2)
                koc = kocpool.tile([P, NO], bf16, tag="koc", name="koc")
                nc.vector.tensor_add(
                    koc[:], ps[:], bias_t["k"][:, o * NO : (o + 1) * NO]
                )
                for h in range(H):
                    prod = prodpool.tile([P, NO], bf16, tag="prod", name="prod")
                    nc.vector.scalar_tensor_tensor(
                        prod[:],
                        qt[i][:, h * DH + c * NO : h * DH + (c + 1) * NO],
                        1.0,
                        koc[:],
                        op0=bypass,
                        op1=mult,
                        accum_out=s2[i][:, (h * H + g2) * 2 + c : (h * H + g2) * 2 + c + 1],
                    )

            fp8_pass(wk8d, x8T, k_sink)

            # ---- softmax (tiny) -> normalized weights en ----
            for i in range(gbt):
                s = smallp.tile([P, 9], f32, tag="s", name="s")
                nc.vector.tensor_reduce(
                    s[:],
                    s2[i][:].rearrange("p (hg two) -> p hg two", two=2),
                    axis=mybir.AxisListType.X,
                    op=add,
                )
                e = smallp.tile([P, 9], f32, tag="e", name="e")
                nc.scalar.activation(e[:], s[:], Exp, scale=ESCALE)
                esum = smallp.tile([P, H], f32, tag="esum", name="esum")
                nc.vector.tensor_reduce(
                    esum[:],
                    e[:].rearrange("p (h g) -> p h g", h=H),
                    axis=mybir.AxisListType.X,
                    op=add,
                )
                rcp = smallp.tile([P, H], f32, tag="rcp", name="rcp")
                nc.vector.reciprocal(rcp[:], esum[:])
                for h in range(H):
                    nc.scalar.mul(
                        en[i][:, h * H : (h + 1) * H],
                        e[:, h * H : (h + 1) * H],
                        rcp[:, h : h + 1],
                    )

            # ---- V pass (bf16) with inline combine on the last chunks ----
            def combine(i, bt, c):
                for h in range(H):
                    ot = otpool.tile([P, NO], f32, tag="ot", name="ot")
                    nc.scalar.mul(
                        ot[:],
                        vt[i][:, 0 * DH + c * NO : 0 * DH + (c + 1) * NO],
                        en[i][:, h * H : h * H + 1],
                    )
                    for g2 in (1, 2):
                        nc.vector.scalar_tensor_tensor(
                            ot[:],
                            vt[i][:, g2 * DH + c * NO : g2 * DH + (c + 1) * NO],
                            en[i][:, h * H + g2 : h * H + g2 + 1],
                            ot[:],
                            op0=mult,
                            op1=add,
                        )
                    nc.scalar.dma_start(
                        outd[bt * P : bt * P + P, h * DH + c * NO : h * DH + (c + 1) * NO],
                        ot[:],
                    )

            for o in range(NOT):
                ps = [
                    pspool.tile([P, NO], f32, tag=f"ps{i}", name=f"ps{i}")
                    for i in range(gbt)
                ]
                for kg in range(8):
                    wt = wpool.tile([P, 3, NO], bf16, tag="wv", name="wv")
                    nc.gpsimd.dma_start(wt[:], wv16d[o, kg])
                    for j in range(3):
                        k = kg * 3 + j
                        for i in range(gbt):
                            nc.tensor.matmul(
                                ps[i][:],
                                x16T[i][:, k],
                                wt[:, j],
                                start=(k == 0),
                                stop=(k == KT - 1),
                            )
                for i in range(gbt):
                    nc.vector.tensor_add(
                        vt[i][:, o * NO : (o + 1) * NO],
                        ps[i][:],
                        bias_t["v"][:, o * NO : (o + 1) * NO],
                    )
                if o == NOT - 2:
                    for i, bt in enumerate(bts):
                        combine(i, bt, 0)
                elif o == NOT - 1:
                    for i, bt in enumerate(bts):
                        combine(i, bt, 1)

    nc.compile()
    return nc


def _get_nc(bs=B // NCORES, gbt=GBT):
    key = (bs, gbt)
    if key not in _CACHE:
        _CACHE[key] = _build(bs, gbt)
    return _CACHE[key]


def _prep_weights(Wq, Wk, Wv, bq, bk, bv):
    """Pre-tile weights.

    fp8 Q/K: w8[o, kg, p, j, i, n] = (WS*W.T)[((kg*3+j)*2+i)*128+p, o*512+n]
    bf16 V:  wv[o, kg, p, j, n]    =      Wv.T[(kg*3+j)*128+p,     o*512+n]
    """
    ws = {}
    for nm, W in (("q", Wq), ("k", Wk)):
        wt = (np.asarray(W, dtype=np.float32).T * np.float32(WS)).astype(E4)
        w6 = wt.reshape(4, 3, 2, P, NOT, NO).transpose(4, 0, 3, 1, 2, 5)
        ws[nm] = np.ascontiguousarray(w6)
    wtv = np.asarray(Wv, dtype=np.float32).T.astype(BF)
    wv5 = wtv.reshape(8, 3, P, NOT, NO).transpose(3, 0, 2, 1, 4)
    ws["v"] = np.ascontiguousarray(wv5)

    bb = {}
    for nm, b, sc in (("q", bq, WS), ("k", bk, WS), ("v", bv, 1.0)):
        bs_ = (np.asarray(b, dtype=np.float32) * np.float32(sc)).astype(BF)
        bb[nm] = np.ascontiguousarray(np.broadcast_to(bs_, (P, D)))
    return ws, bb


def _prep_act8(a, bs):
    """fp8 DoubleRow: a8[bt, p, k8, i, b] = a[bt*128+b, (k8*2+i)*128+p]."""
    nbt = bs // P
    a8 = a.astype(E4).reshape(nbt, P, K8, 2, P).transpose(0, 4, 2, 3, 1)
    return np.ascontiguousarray(a8)


def _prep_act16(a, bs):
    """bf16: a16[bt, p, k, b] = a[bt*128+b, k*128+p]."""
    nbt = bs // P
    a16 = a.astype(BF).reshape(nbt, P, KT, P).transpose(0, 3, 2, 1)
    return np.ascontiguousarray(a16)


def _in_maps(x, sa, ws, bb, bs):
    maps = []
    for c in range(NCORES):
        r0 = c * bs
        maps.append(
            {
                "sa8": _prep_act8(sa[r0 : r0 + bs], bs),
                "x8": _prep_act8(x[r0 : r0 + bs], bs),
                "x16": _prep_act16(x[r0 : r0 + bs], bs),
                "wq8": ws["q"],
                "wk8": ws["k"],
                "wv16": ws["v"],
                "bq128": bb["q"],
                "bk128": bb["k"],
                "bv": bb["v"],
            }
        )
    return maps


def kernel(x, synthetic_attributes, Wq, bq, Wk, bk, Wv, bv, **_ignored):
    from concourse import bass_utils

    x = np.asarray(x, dtype=np.float32)
    sa = np.asarray(synthetic_attributes, dtype=np.float32)
    bs = x.shape[0] // NCORES

    ws, bb = _prep_weights(Wq, Wk, Wv, bq, bk, bv)
    nc = _get_nc(bs=bs)
    in_maps = _in_maps(x, sa, ws, bb, bs)

    res = bass_utils.run_bass_kernel_spmd(nc, in_maps, core_ids=list(range(NCORES)))
    out = np.concatenate([res.results[c]["out"] for c in range(NCORES)], axis=0)
    return out


# revision 6
# speedup vs baseline: 1.9789x; 1.0343x over previous
"""Trainium2 Bass kernel for AttributeAttentionModule.

y = attention over heads of QKV projections:
  Q = sa @ Wq.T + bq ; K = x @ Wk.T + bk ; V = x @ Wv.T + bv   (all [B, D])
  per-sample scores[h,g] = Q_h . K_g / 32 ; softmax over g ; out_h = sum_g w_hg V_g

Data-parallel over 8 NeuronCores (batch sharded). Q/K projections run in
fp8 e4m3 with DoubleRow perf mode (2 rows/cycle, 256-deep contraction per
instruction); V runs in bf16 (fp8 noise in V would pass straight through the
softmax-convex combination, Q/K noise is damped by it). Weights are prescaled
by 128 for fp8 (raw weights sit below e4m3's normal range); the 1/128^2 is
folded into the softmax exp scale. Attention is fully fused in SBUF: Q is
kept per group of 4 batch-tiles, scores are accumulated inline as each K
PSUM chunk lands, V is kept bf16 per group and combined into the output
right after its last column chunks, so Q/K/V never round-trip through DRAM.
"""

import os
import sys

for _p in ("/opt/trn_rl_repo", "/root/.axon_site/_ro/trn_rl_repo"):
    if os.path.isdir(_p) and _p not in sys.path:
        sys.path.append(_p)

import numpy as np
import ml_dtypes
from contextlib import ExitStack

B = 16384
D = 3072
H = 3
DH = D // H          # 1024
NCORES = 8
P = 128              # partition tile
NO = 512             # matmul moving free dim (one PSUM bank of fp32)
NOT = D // NO        # 6 output-column tiles
KT = D // P          # 24 contraction tiles of 128
K8 = KT // 2         # 12 fp8 DoubleRow contraction tiles of 256
WS = 128.0           # fp8 weight prescale (power of two, exact)
ESCALE = 1.0 / (32.0 * WS * WS)  # softmax exp scale: 1/sqrt(dh) / WS^2
GBT = 4              # batch tiles per weight-streaming group

E4 = ml_dtypes.float8_e4m3
BF = ml_dtypes.bfloat16

_CACHE = {}


def _build(bs=B // NCORES, gbt=GBT):
    import concourse.bass as bass
    import concourse.tile as tile
    from concourse import bacc, mybir

    f32 = mybir.dt.float32
    f8 = mybir.dt.float8e4
    bf16 = mybir.dt.bfloat16
    mult = mybir.AluOpType.mult
    add = mybir.AluOpType.add
    bypass = mybir.AluOpType.bypass
    Exp = mybir.ActivationFunctionType.Exp
    DR = mybir.MatmulPerfMode.DoubleRow

    nbt = bs // P        # batch tiles per core
    ng = nbt // gbt      # weight-stream groups

    nc = bacc.Bacc(
        "TRN2", target_bir_lowering=False, debug=False, num_devices=NCORES
    )

    # pre-tiled inputs (see kernel() for host layouts)
    sa8d = nc.dram_tensor("sa8", [nbt, P, K8, 2, P], f8, kind="ExternalInput").ap()
    x8d = nc.dram_tensor("x8", [nbt, P, K8, 2, P], f8, kind="ExternalInput").ap()
    x16d = nc.dram_tensor("x16", [nbt, P, KT, P], bf16, kind="ExternalInput").ap()
    wq8d = nc.dram_tensor("wq8", [NOT, 4, P, 3, 2, NO], f8, kind="ExternalInput").ap()
    wk8d = nc.dram_tensor("wk8", [NOT, 4, P, 3, 2, NO], f8, kind="ExternalInput").ap()
    wv16d = nc.dram_tensor("wv16", [NOT, 8, P, 3, NO], bf16, kind="ExternalInput").ap()
    bqd = nc.dram_tensor("bq128", [P, D], bf16, kind="ExternalInput").ap()
    bkd = nc.dram_tensor("bk128", [P, D], bf16, kind="ExternalInput").ap()
    bvd = nc.dram_tensor("bv", [P, D], bf16, kind="ExternalInput").ap()
    outd = nc.dram_tensor("out", [bs, D], f32, kind="ExternalOutput").ap()

    with tile.TileContext(nc) as tc, ExitStack() as ctx:
        sapool = ctx.enter_context(tc.tile_pool(name="sapool", bufs=1))
        x8pool = ctx.enter_context(tc.tile_pool(name="x8pool", bufs=1))
        x16pool = ctx.enter_context(tc.tile_pool(name="x16pool", bufs=1))
        qpool = ctx.enter_context(tc.tile_pool(name="qpool", bufs=1))
        accpool = ctx.enter_context(tc.tile_pool(name="accpool", bufs=1))
        wpool = ctx.enter_context(tc.tile_pool(name="wpool", bufs=4))
        bpool = ctx.enter_context(tc.tile_pool(name="bpool", bufs=1))
        pspool = ctx.enter_context(tc.tile_pool(name="psum", bufs=2, space="PSUM"))
        kocpool = ctx.enter_context(tc.tile_pool(name="kocp", bufs=3))
        prodpool = ctx.enter_context(tc.tile_pool(name="prodp", bufs=2))
        smallp = ctx.enter_context(tc.tile_pool(name="smallp", bufs=1))

        bias_loaded = False
        bias_t = {}
        pre_wq = None  # next group's o=0 Q-weight tiles, prefetched in V pass

        for g in range(ng):
            bts = [g * gbt + i for i in range(gbt)]

            # activation loads for this group (sync queue: independent of
            # the weight stream on gpsimd so they land during prior passes)
            saT, x8T, x16T = [], [], []
            for i, bt in enumerate(bts):
                t = sapool.tile([P, K8, 2, P], f8, tag=f"sa{i}", name=f"sa{i}")
                nc.sync.dma_start(t[:], sa8d[bt])
                saT.append(t)
            if not bias_loaded:
                # after the first group's Q stationaries so they don't delay
                # the first matmul; they are only needed at the first q_sink
                bias_loaded = True
                for nm, src in (("q", bqd), ("k", bkd), ("v", bvd)):
                    t = bpool.tile([P, D], bf16, tag=f"b{nm}", name=f"b{nm}")
                    nc.sync.dma_start(t[:], src[:])
                    bias_t[nm] = t
            for i, bt in enumerate(bts):
                t = x8pool.tile([P, K8, 2, P], f8, tag=f"x8{i}", name=f"x8{i}")
                nc.sync.dma_start(t[:], x8d[bt])
                x8T.append(t)
            for i, bt in enumerate(bts):
                t = x16pool.tile([P, KT, P], bf16, tag=f"x16{i}", name=f"x16{i}")
                nc.sync.dma_start(t[:], x16d[bt])
                x16T.append(t)

            qt = [
                qpool.tile([P, D], bf16, tag=f"q{i}", name=f"q{i}")
                for i in range(gbt)
            ]
            acc = [
                accpool.tile([P, D], f32, tag=f"acc{i}", name=f"acc{i}")
                for i in range(gbt)
            ]
            s2 = [
                smallp.tile([P, 18], f32, tag=f"s2_{i}", name=f"s2_{i}")
                for i in range(gbt)
            ]
            en = [
                smallp.tile([P, 9], f32, tag=f"en_{i}", name=f"en_{i}")
                for i in range(gbt)
            ]

            def fp8_pass(wTd, acts, sink, pre=None):
                """One fp8 DoubleRow projection sweep; sink(i, o, ps) consumes
                each finished PSUM chunk. pre = prefetched o=0 weight tiles."""
                for o in range(NOT):
                    ps = [
                        pspool.tile([P, NO], f32, tag=f"ps{i}", name=f"ps{i}")
                        for i in range(gbt)
                    ]
                    for kg in range(4):
                        if o == 0 and pre is not None:
                            wt = pre[kg]
                        else:
                            wt = wpool.tile([P, 3, 2, NO], f8, tag="w8", name="w8")
                            nc.gpsimd.dma_start(wt[:], wTd[o, kg])
                        for j in range(3):
                            k8 = kg * 3 + j
                            for i in range(gbt):
                                nc.tensor.matmul(
                                    ps[i][:],
                                    acts[i][:, k8],
                                    wt[:, j],
                                    start=(k8 == 0),
                                    stop=(k8 == K8 - 1),
                                    perf_mode=DR,
                                )
                    for i in range(gbt):
                        sink(i, o, ps[i])

            # ---- Q pass: oc = ps + 128*bq -> qt (bf16) ----
            def q_sink(i, o, ps):
                nc.vector.tensor_add(
                    qt[i][:, o * NO : (o + 1) * NO],
                    ps[:],
                    bias_t["q"][:, o * NO : (o + 1) * NO],
                )

            fp8_pass(wq8d, saT, q_sink, pre=pre_wq)
            pre_wq = None

            # ---- K pass: inline score partials, K never stored ----
            def k_sink(i, o, ps):
                g2, c = divmod(o, 2)
                koc = kocpool.tile([P, NO], bf16, tag="koc", name="koc")
                nc.vector.tensor_add(
                    koc[:], ps[:], bias_t["k"][:, o * NO : (o + 1) * NO]
                )
                for h in range(H):
                    prod = prodpool.tile([P, NO], bf16, tag="prod", name="prod")
                    nc.vector.scalar_tensor_tensor(
                        prod[:],
                        qt[i][:, h * DH + c * NO : h * DH + (c + 1) * NO],
                        1.0,
                        koc[:],
                        op0=bypass,
                        op1=mult,
                        accum_out=s2[i][:, (h * H + g2) * 2 + c : (h * H + g2) * 2 + c + 1],
                    )

            fp8_pass(wk8d, x8T, k_sink)

            # ---- softmax (tiny) -> normalized weights en ----
            for i in range(gbt):
                s = smallp.tile([P, 9], f32, tag="s", name="s")
                nc.vector.tensor_reduce(
                    s[:],
                    s2[i][:].rearrange("p (hg two) -> p hg two", two=2),
                    axis=mybir.AxisListType.X,
                    op=add,
                )
                e = smallp.tile([P, 9], f32, tag="e", name="e")
                nc.scalar.activation(e[:], s[:], Exp, scale=ESCALE)
                esum = smallp.tile([P, H], f32, tag="esum", name="esum")
                nc.vector.tensor_reduce(
                    esum[:],
                    e[:].rearrange("p (h g) -> p h g", h=H),
                    axis=mybir.AxisListType.X,
                    op=add,
                )
                rcp = smallp.tile([P, H], f32, tag="rcp", name="rcp")
                nc.vector.reciprocal(rcp[:], esum[:])
                for h in range(H):
                    nc.scalar.mul(
                        en[i][:, h * H : (h + 1) * H],
                        e[:, h * H : (h + 1) * H],
                        rcp[:, h : h + 1],
                    )

            # ---- V pass (bf16): each PSUM chunk (head g2, col-half c) is
            # combined immediately into the per-head output accumulators;
            # after the g2==2 chunk the (h, c) slices are final -> DMA out.
            for o in range(NOT):
                g2, c = divmod(o, 2)
                ps = [
                    pspool.tile([P, NO], f32, tag=f"ps{i}", name=f"ps{i}")
                    for i in range(gbt)
                ]
                for kg in range(8):
                    wt = wpool.tile([P, 3, NO], bf16, tag="wv", name="wv")
                    nc.gpsimd.dma_start(wt[:], wv16d[o, kg])
                    for j in range(3):
                        k = kg * 3 + j
                        for i in range(gbt):
                            nc.tensor.matmul(
                                ps[i][:],
                                x16T[i][:, k],
                                wt[:, j],
                                start=(k == 0),
                                stop=(k == KT - 1),
                            )
                # prefetch next group's first Q-weight tiles once the V weight
                # stream is fully queued, so the V->Q boundary doesn't stall
                if o == 0 and g < ng - 1:
                    pre_wq = []
                    for kg in range(4):
                        wt = wpool.tile([P, 3, 2, NO], f8, tag="w8", name="w8")
                        nc.gpsimd.dma_start(wt[:], wq8d[0, kg])
                        pre_wq.append(wt)
                for i, bt in enumerate(bts):
                    voc = kocpool.tile([P, NO], bf16, tag="koc", name="koc")
                    nc.vector.tensor_add(
                        voc[:], ps[i][:], bias_t["v"][:, o * NO : (o + 1) * NO]
                    )
                    for h in range(H):
                        asl = acc[i][:, h * DH + c * NO : h * DH + (c + 1) * NO]
                        if g2 == 0:
                            nc.scalar.mul(asl, voc[:], en[i][:, h * H : h * H + 1])
                        else:
                            nc.vector.scalar_tensor_tensor(
                                asl,
                                voc[:],
                                en[i][:, h * H + g2 : h * H + g2 + 1],
                                asl,
                                op0=mult,
                                op1=add,
                            )
                        if g2 == 2:
                            nc.scalar.dma_start(
                                outd[
                                    bt * P : bt * P + P,
                                    h * DH + c * NO : h * DH + (c + 1) * NO,
                                ],
                                asl,
                            )

    nc.compile()
    return nc


def _get_nc(bs=B // NCORES, gbt=GBT):
    key = (bs, gbt)
    if key not in _CACHE:
        _CACHE[key] = _build(bs, gbt)
    return _CACHE[key]


def _prep_weights(Wq, Wk, Wv, bq, bk, bv):
    """Pre-tile weights.

    fp8 Q/K: w8[o, kg, p, j, i, n] = (WS*W.T)[((kg*3+j)*2+i)*128+p, o*512+n]
    bf16 V:  wv[o, kg, p, j, n]    =      Wv.T[(kg*3+j)*128+p,     o*512+n]
    """
    ws = {}
    for nm, W in (("q", Wq), ("k", Wk)):
        wt = (np.asarray(W, dtype=np.float32).T * np.float32(WS)).astype(E4)
        w6 = wt.reshape(4, 3, 2, P, NOT, NO).transpose(4, 0, 3, 1, 2, 5)
        ws[nm] = np.ascontiguousarray(w6)
    wtv = np.asarray(Wv, dtype=np.float32).T.astype(BF)
    wv5 = wtv.reshape(8, 3, P, NOT, NO).transpose(3, 0, 2, 1, 4)
    ws["v"] = np.ascontiguousarray(wv5)

    bb = {}
    for nm, b, sc in (("q", bq, WS), ("k", bk, WS), ("v", bv, 1.0)):
        bs_ = (np.asarray(b, dtype=np.float32) * np.float32(sc)).astype(BF)
        bb[nm] = np.ascontiguousarray(np.broadcast_to(bs_, (P, D)))
    return ws, bb


def _prep_act8(a, bs):
    """fp8 DoubleRow: a8[bt, p, k8, i, b] = a[bt*128+b, (k8*2+i)*128+p]."""
    nbt = bs // P
    a8 = a.astype(E4).reshape(nbt, P, K8, 2, P).transpose(0, 4, 2, 3, 1)
    return np.ascontiguousarray(a8)


def _prep_act16(a, bs):
    """bf16: a16[bt, p, k, b] = a[bt*128+b, k*128+p]."""
    nbt = bs // P
    a16 = a.astype(BF).reshape(nbt, P, KT, P).transpose(0, 3, 2, 1)
    return np.ascontiguousarray(a16)


def _in_maps(x, sa, ws, bb, bs):
    maps = []
    for c in range(NCORES):
        r0 = c * bs
        maps.append(
            {
                "sa8": _prep_act8(sa[r0 : r0 + bs], bs),
                "x8": _prep_act8(x[r0 : r0 + bs], bs),
                "x16": _prep_act16(x[r0 : r0 + bs], bs),
                "wq8": ws["q"],
                "wk8": ws["k"],
                "wv16": ws["v"],
                "bq128": bb["q"],
                "bk128": bb["k"],
                "bv": bb["v"],
            }
        )
    return maps


def kernel(x, synthetic_attributes, Wq, bq, Wk, bk, Wv, bv, **_ignored):
    from concourse import bass_utils

    x = np.asarray(x, dtype=np.float32)
    sa = np.asarray(synthetic_attributes, dtype=np.float32)
    bs = x.shape[0] // NCORES

    ws, bb = _prep_weights(Wq, Wk, Wv, bq, bk, bv)
    nc = _get_nc(bs=bs)
    in_maps = _in_maps(x, sa, ws, bb, bs)

    res = bass_utils.run_bass_kernel_spmd(nc, in_maps, core_ids=list(range(NCORES)))
    out = np.concatenate([res.results[c]["out"] for c in range(NCORES)], axis=0)
    return out
